# revision 5
# baseline (speedup 1.0000x reference)
"""Trainium2 Bass kernel for a 2-layer LSTM decoder (B=512, T=128, H=1024).

Strategy (data-parallel over batch, per sharding hint):
  - 8 cores, 64 batch rows each; weights replicated, recurrence device-local.
  - All weights cast to bf16 and kept resident in SBUF for the whole T loop.
  - Gates are computed with batch as the matmul *stationary* operand
    (lhsT = h^T chunks, [K=128, M=64]) and weights streaming ([128, 512]).
    Two 512-wide gate chunks are computed concurrently via PE column tiling
    (outputs to PSUM partitions 0-63 and 64-127), giving a "stacked"
    [128, 512] gate tile layout that keeps all 128 lanes busy for the
    activation/elementwise work as well.
  - h is transposed back each step with PE-mode transposes (4x [128,128]).
  - The output projection is an 8-chunk matmul with M=1 producing out^T
    directly as a [1, 64] row, which feeds the next step's input chunk and
    is DMA'd to DRAM; the MSE-loss tail is computed on the host in fp32.
"""

import os

import numpy as np
import ml_dtypes

import concourse.bass as bass
import concourse.tile as tile
import concourse.mybir as mybir

BF16 = ml_dtypes.bfloat16
N_CORES = 8
B, T_FULL, H = 512, 128, 1024
BL = B // N_CORES  # 64 local batch rows
AF = mybir.ActivationFunctionType
DT = mybir.dt

_T = int(os.environ.get("LSTM_KERNEL_T", str(T_FULL)))


def _split_multi_waits(nc):
    """walrus in this container supports only ONE sync wait per instruction.
    Move extra waits onto preceding same-engine NOPs (engine FIFO makes this
    semantically identical)."""
    for f in nc.m.functions:
        for bb in f.blocks:
            new = []
            for ins in bb.instructions:
                si = ins.sync_info
                if si is not None and si.on_wait and len(si.on_wait) > 1:
                    waits = list(si.on_wait)
                    for w in waits[:-1]:
                        nop = mybir.InstNoOp(
                            name=nc.get_next_instruction_name(), ins=[], outs=[]
                        )
                        nop.engine = ins.engine
                        nop.sync_info = mybir.SyncInfo(on_wait=[w], on_update=[])
                        nc.register_instruction(nop)
                        new.append(nop)
                    si.on_wait = [waits[-1]]
                new.append(ins)
            bb.instructions = new


def _build_program(t_steps):
    nc = bass.Bass(dynamic_dma_scratch_size=512)

    w0_d = nc.dram_tensor("W0", [128, 9, 4096], DT.bfloat16, kind="ExternalInput")
    w1_d = nc.dram_tensor("W1", [128, 17, 4096], DT.bfloat16, kind="ExternalInput")
    wout_d = nc.dram_tensor("WOUT", [128, 8], DT.bfloat16, kind="ExternalInput")
    idt_d = nc.dram_tensor("IDT", [128, 128], DT.bfloat16, kind="ExternalInput")
    ht_d = nc.dram_tensor("HT", [128, 8, 128], DT.bfloat16, kind="ExternalInput")
    cs_d = nc.dram_tensor("CS", [128, 2, 512], DT.float32, kind="ExternalInput")
    bout_d = nc.dram_tensor("BOUT", [1, 1], DT.float32, kind="ExternalInput")
    xinit_d = nc.dram_tensor("XINIT", [2, BL], DT.bfloat16, kind="ExternalInput")
    outd = nc.dram_tensor("OUTD", [T_FULL, BL], DT.float32, kind="ExternalOutput")

    with tile.TileContext(nc) as tc:
        with (
            tc.tile_pool(name="const", bufs=1) as const,
            tc.tile_pool(name="psum", bufs=8, space="PSUM") as psum,
            tc.tile_pool(name="tmp", bufs=3) as tmp,
            tc.tile_pool(name="drow", bufs=2) as drowp,
        ):
            w0 = const.tile([128, 9, 4096], DT.bfloat16)
            w1 = const.tile([128, 17, 4096], DT.bfloat16)
            wout = const.tile([128, 8], DT.bfloat16)
            idt = const.tile([128, 128], DT.bfloat16)
            ht = const.tile([128, 8, 128], DT.bfloat16)
            cs = const.tile([128, 2, 512], DT.float32)
            bout = const.tile([1, 1], DT.float32)
            xch = const.tile([2, BL], DT.bfloat16)
            ones = const.tile([1, BL], DT.bfloat16)
            hs0 = const.tile([128, 512], DT.bfloat16)
            hs1 = const.tile([128, 512], DT.bfloat16)

            nc.sync.dma_start(w0[:], w0_d[:])
            nc.sync.dma_start(w1[:], w1_d[:])
            nc.sync.dma_start(wout[:], wout_d[:])
            nc.sync.dma_start(idt[:], idt_d[:])
            nc.sync.dma_start(ht[:], ht_d[:])
            nc.sync.dma_start(cs[:], cs_d[:])
            nc.sync.dma_start(bout[:], bout_d[:])

            nc.sync.dma_start(xch[:], xinit_d[:])
            nc.vector.memset(ones[:], 1.0)

            # h^T chunk APs: layer-l chunk k covers hidden dims [128k, 128k+128)
            # block p of ht holds cols 0:64 = chunk p, cols 64:128 = chunk 4+p.
            def h_chunk(layer, k):
                base = 4 * layer
                if k < 4:
                    return ht[:, base + k, 0:BL]
                return ht[:, base + (k - 4), BL : 2 * BL]

            GFUNC = [AF.Sigmoid, AF.Sigmoid, AF.Tanh, AF.Sigmoid]

            for t in range(t_steps):
                # ---- L0 gate matmuls: gates0 = [h0; x, 1] @ [Whh0; Wih0, b0]
                p0 = [psum.tile([128, 512], DT.float32, tag="bank", name=f"p0_{t}_{g}") for g in range(4)]
                for k in range(9):
                    lhsT = h_chunk(0, k) if k < 8 else xch[0:2, :]
                    for g in range(4):
                        for hf in range(2):
                            s = 1024 * g + 512 * hf
                            rhs = (
                                w0[:, k, s : s + 512]
                                if k < 8
                                else w0[0:2, 8, s : s + 512]
                            )
                            nc.tensor.matmul(
                                p0[g][BL * hf : BL * (hf + 1), :],
                                lhsT,
                                rhs,
                                start=(k == 0),
                                stop=(k == 8),
                            )

                # ---- L1 early part: h1(t-1) @ Whh1 + bias (independent of L0)
                p1 = [psum.tile([128, 512], DT.float32, tag="bank", name=f"p1_{t}_{g}") for g in range(4)]
                for k in range(8):
                    lhsT = h_chunk(1, k)
                    for g in range(4):
                        for hf in range(2):
                            s = 1024 * g + 512 * hf
                            nc.tensor.matmul(
                                p1[g][BL * hf : BL * (hf + 1), :],
                                lhsT,
                                w1[:, 8 + k, s : s + 512],
                                start=(k == 0),
                                stop=False,
                            )
                for g in range(4):
                    for hf in range(2):
                        s = 1024 * g + 512 * hf
                        nc.tensor.matmul(
                            p1[g][BL * hf : BL * (hf + 1), :],
                            ones[0:1, :],
                            w1[0:1, 16, s : s + 512],
                            start=False,
                            stop=False,
                        )

                # ---- L0 activations + elementwise
                gs = tmp.tile([128, 512], DT.float32, tag="tmp")
                nc.scalar.activation(gs[:], p0[2][:], AF.Tanh)
                for g in (0, 1, 3):
                    nc.scalar.activation(p0[g][:], p0[g][:], AF.Sigmoid)
                t1 = tmp.tile([128, 512], DT.float32, tag="tmp")
                cs0 = cs[:, 0, :]
                nc.vector.tensor_mul(t1[:], p0[0][:], gs[:])
                nc.vector.tensor_mul(cs0, p0[1][:], cs0)
                nc.vector.tensor_add(cs0, cs0, t1[:])
                thc = tmp.tile([128, 512], DT.float32, tag="tmp")
                nc.scalar.activation(thc[:], cs0, AF.Tanh)
                nc.vector.tensor_mul(hs0[:], p0[3][:], thc[:])

                # ---- transpose h0 -> ht blocks 0..3
                trp0 = psum.tile([128, 4, 128], DT.bfloat16, tag="bank")
                for p in range(4):
                    nc.tensor.transpose(
                        trp0[:, p, :], hs0[:, 128 * p : 128 * (p + 1)], idt[:]
                    )
                for p in range(4):
                    nc.vector.tensor_copy(ht[:, p, :], trp0[:, p, :])

                # ---- L1 late part: h0(t) @ Wih1
                for k in range(8):
                    lhsT = h_chunk(0, k)
                    for g in range(4):
                        for hf in range(2):
                            s = 1024 * g + 512 * hf
                            nc.tensor.matmul(
                                p1[g][BL * hf : BL * (hf + 1), :],
                                lhsT,
                                w1[:, k, s : s + 512],
                                start=False,
                                stop=(k == 7),
                            )

                # ---- L1 activations + elementwise
                gs1 = tmp.tile([128, 512], DT.float32, tag="tmp")
                nc.scalar.activation(gs1[:], p1[2][:], AF.Tanh)
                for g in (0, 1, 3):
                    nc.scalar.activation(p1[g][:], p1[g][:], AF.Sigmoid)
                t2 = tmp.tile([128, 512], DT.float32, tag="tmp")
                cs1 = cs[:, 1, :]
                nc.vector.tensor_mul(t2[:], p1[0][:], gs1[:])
                nc.vector.tensor_mul(cs1, p1[1][:], cs1)
                nc.vector.tensor_add(cs1, cs1, t2[:])
                thc1 = tmp.tile([128, 512], DT.float32, tag="tmp")
                nc.scalar.activation(thc1[:], cs1, AF.Tanh)
                nc.vector.tensor_mul(hs1[:], p1[3][:], thc1[:])

                # ---- transpose h1 -> ht blocks 4..7
                trp1 = psum.tile([128, 4, 128], DT.bfloat16, tag="bank")
                for p in range(4):
                    nc.tensor.transpose(
                        trp1[:, p, :], hs1[:, 128 * p : 128 * (p + 1)], idt[:]
                    )
                for p in range(4):
                    nc.vector.tensor_copy(ht[:, 4 + p, :], trp1[:, p, :])

                # ---- out^T = W_out @ h1^T : [1, 64] via M=1 matmuls
                dps = psum.tile([1, BL], DT.float32, tag="bank")
                for k in range(8):
                    nc.tensor.matmul(
                        dps[0:1, :],
                        wout[:, k : k + 1],
                        h_chunk(1, k),
                        start=(k == 0),
                        stop=(k == 7),
                    )

                # ---- tail: out rows (fp32 to DRAM, bf16 to next x)
                drow = drowp.tile([1, BL], DT.float32, tag="drow")
                nc.scalar.activation(
                    drow[:], dps[0:1, :], AF.Identity, bias=bout[0:1, 0:1]
                )
                nc.scalar.activation(
                    xch[0:1, :], dps[0:1, :], AF.Identity, bias=bout[0:1, 0:1]
                )
                nc.sync.dma_start(outd[t : t + 1, :], drow[:])

    _split_multi_waits(nc)
    return nc


# ---------------------------------------------------------------------------
# host side


def _stack_batch(x):  # [64, 1024] -> stacked [128, 512]
    return x.reshape(BL, 2, 512).transpose(1, 0, 2).reshape(128, 512)


def _ht_blocks(h):  # [64, 1024] -> [128(j), 4(p), 128(r)]
    x = h.reshape(BL, 2, 4, 128)  # [b, half, p, j]
    return x.transpose(3, 2, 1, 0).reshape(128, 4, 128)


def _prep_shared(inp):
    w0 = np.zeros((128, 9, 4096), dtype=np.float32)
    w0[:, :8, :] = inp["W_hh0"].T.reshape(8, 128, 4096).transpose(1, 0, 2)
    w0[0, 8, :] = inp["W_ih0"][:, 0]
    w0[1, 8, :] = inp["b_ih0"] + inp["b_hh0"]

    w1 = np.zeros((128, 17, 4096), dtype=np.float32)
    w1[:, 0:8, :] = inp["W_ih1"].T.reshape(8, 128, 4096).transpose(1, 0, 2)
    w1[:, 8:16, :] = inp["W_hh1"].T.reshape(8, 128, 4096).transpose(1, 0, 2)
    w1[0, 16, :] = inp["b_ih1"] + inp["b_hh1"]

    wo = np.ascontiguousarray(inp["W_out"][0].reshape(8, 128).T)
    return {
        "W0": w0.astype(BF16),
        "W1": w1.astype(BF16),
        "WOUT": wo.astype(BF16),
        "IDT": np.eye(128, dtype=BF16),
        "XINIT": np.concatenate(
            [np.zeros((1, BL), BF16), np.ones((1, BL), BF16)], axis=0
        ),
        "BOUT": inp["b_out"].reshape(1, 1).astype(np.float32),
    }


def _prep_core(inp, c):
    sl = slice(BL * c, BL * (c + 1))
    ht = np.zeros((128, 8, 128), dtype=np.float32)
    ht[:, 0:4, :] = _ht_blocks(np.asarray(inp["h0"][0, sl]))
    ht[:, 4:8, :] = _ht_blocks(np.asarray(inp["h0"][1, sl]))
    cs = np.zeros((128, 2, 512), dtype=np.float32)
    cs[:, 0, :] = _stack_batch(np.asarray(inp["c0"][0, sl]))
    cs[:, 1, :] = _stack_batch(np.asarray(inp["c0"][1, sl]))
    return {"HT": ht.astype(BF16), "CS": cs}


_RUNNER = {}


def _get_runner(t_steps):
    """Build the bass program once per process and return a cached callable
    mapping per-core input dicts -> per-core OUTD arrays."""
    if t_steps in _RUNNER:
        return _RUNNER[t_steps]

    import jax
    from jax.sharding import Mesh, PartitionSpec
    from jax.experimental.shard_map import shard_map
    from concourse import bass2jax
    from concourse._compat import axon_active

    nc = _build_program(t_steps)

    if not axon_active():
        from concourse.bass_utils import run_bass_kernel_spmd

        def run_native(in_maps):
            res = run_bass_kernel_spmd(nc, in_maps, list(range(N_CORES)))
            return [r["OUTD"] for r in res.results]

        _RUNNER[t_steps] = run_native
        return run_native

    bass2jax.install_neuronx_cc_hook()

    partition_name = nc.partition_id_tensor.name if nc.partition_id_tensor else None
    in_names = []
    out_names = []
    out_avals = []
    zero_outs = []
    for alloc in nc.m.functions[0].allocations:
        if not isinstance(alloc, mybir.MemoryLocationSet):
            continue
        name = alloc.memorylocations[0].name
        if alloc.kind == "ExternalInput":
            if name != partition_name:
                in_names.append(name)
        elif alloc.kind == "ExternalOutput":
            out_names.append(name)
            shape = tuple(alloc.tensor_shape)
            dtype = mybir.dt.np(alloc.dtype)
            out_avals.append(jax.core.ShapedArray(shape, dtype))
            zero_outs.append(np.zeros(shape, dtype))
    n_params = len(in_names)
    n_outs = len(out_avals)
    all_names = in_names + out_names
    if partition_name is not None:
        all_names = all_names + [partition_name]
    donate = tuple(range(n_params, n_params + n_outs))

    def _body(*args):
        operands = list(args)
        if partition_name is not None:
            operands.append(bass2jax.partition_id_tensor())
        outs = bass2jax._bass_exec_p.bind(
            *operands,
            out_avals=tuple(out_avals),
            in_names=tuple(all_names),
            out_names=tuple(out_names),
            lowering_input_output_aliases=(),
            sim_require_finite=True,
            sim_require_nnan=True,
            nc=nc,
        )
        return tuple(outs)

    devices = jax.devices()[:N_CORES]
    mesh = Mesh(np.asarray(devices), ("core",))
    sharded = jax.jit(
        shard_map(
            _body,
            mesh=mesh,
            in_specs=(PartitionSpec("core"),) * (n_params + n_outs),
            out_specs=(PartitionSpec("core"),) * n_outs,
            check_rep=False,
        ),
        donate_argnums=donate,
        keep_unused=True,
    )

    def run(in_maps):
        concat_in = [
            np.concatenate([np.asarray(in_maps[c][nm]) for c in range(N_CORES)], axis=0)
            for nm in in_names
        ]
        concat_zero = [
            np.concatenate([z] * N_CORES, axis=0) for z in zero_outs
        ]
        out_arrs = sharded(*concat_in, *concat_zero)
        outs = []
        for i, nm in enumerate(out_names):
            full = np.asarray(out_arrs[i])
            outs.append(np.split(full, N_CORES, axis=0))
        # outs[i][c] ; only OUTD
        return [outs[0][c] for c in range(N_CORES)]

    _RUNNER[t_steps] = run
    return run


def kernel(**inputs):
    inp = {k: np.asarray(v) for k, v in inputs.items()}
    for k in ("W_ih0", "W_hh0", "b_ih0", "b_hh0", "W_ih1", "W_hh1", "b_ih1",
              "b_hh1", "W_out", "b_out", "h0", "c0", "outputs"):
        assert k in inp, f"missing input {k}"

    shared = _prep_shared(inp)
    in_maps = []
    for c in range(N_CORES):
        m = dict(shared)
        m.update(_prep_core(inp, c))
        in_maps.append(m)

    run = _get_runner(_T)
    outs = run(in_maps)  # list of [T_FULL, BL] fp32 per core

    out_all = np.concatenate(outs, axis=1)  # [T, B]
    targets = np.asarray(inp["outputs"]).T.astype(np.float32)  # [T, B]
    d = out_all[:_T].astype(np.float64) - targets[:_T].astype(np.float64)
    loss = np.sum(np.mean(d * d, axis=1))
    return np.float32(loss)


# revision 6
# speedup vs baseline: 723.1978x; 723.1978x over previous
"""Trainium2 Bass kernel for a 2-layer LSTM decoder (B=512, T=128, H=1024).

Strategy (data-parallel over batch, per sharding hint):
  - 8 cores, 64 batch rows each; weights replicated, recurrence device-local.
  - All weights cast to bf16 and kept resident in SBUF for the whole T loop.
  - Gates are computed with batch as the matmul *stationary* operand
    (lhsT = h^T chunks, [K=128, M=64]) and weights streaming ([128, 512]).
    Two 512-wide gate chunks are computed concurrently via PE column tiling
    (outputs to PSUM partitions 0-63 and 64-127), giving a "stacked"
    [128, 512] gate tile layout that keeps all 128 lanes busy for the
    activation/elementwise work as well.
  - h is transposed back each step with PE-mode transposes (4x [128,128]).
  - The output projection is an 8-chunk matmul with M=1 producing out^T
    directly as a [1, 64] row, which feeds the next step's input chunk and
    is DMA'd to DRAM; the MSE-loss tail is computed on the host in fp32.
"""

import os

import numpy as np
import ml_dtypes

import concourse.bass as bass
import concourse.tile as tile
import concourse.mybir as mybir

BF16 = ml_dtypes.bfloat16
N_CORES = 8
B, T_FULL, H = 512, 128, 1024
BL = B // N_CORES  # 64 local batch rows
AF = mybir.ActivationFunctionType
DT = mybir.dt

_T = int(os.environ.get("LSTM_KERNEL_T", str(T_FULL)))


def _split_multi_waits(nc):
    """walrus in this container supports only ONE sync wait per instruction.
    Move extra waits onto preceding same-engine NOPs (engine FIFO makes this
    semantically identical)."""
    for f in nc.m.functions:
        for bb in f.blocks:
            new = []
            for ins in bb.instructions:
                si = ins.sync_info
                if si is not None and si.on_wait and len(si.on_wait) > 1:
                    waits = list(si.on_wait)
                    for w in waits[:-1]:
                        nop = mybir.InstNoOp(
                            name=nc.get_next_instruction_name(), ins=[], outs=[]
                        )
                        nop.engine = ins.engine
                        nop.sync_info = mybir.SyncInfo(on_wait=[w], on_update=[])
                        nc.register_instruction(nop)
                        new.append(nop)
                    si.on_wait = [waits[-1]]
                new.append(ins)
            bb.instructions = new


def _build_program(t_steps):
    nc = bass.Bass(dynamic_dma_scratch_size=512)

    w0_d = nc.dram_tensor("W0", [128, 9, 4096], DT.bfloat16, kind="ExternalInput")
    w1_d = nc.dram_tensor("W1", [128, 17, 4096], DT.bfloat16, kind="ExternalInput")
    wout_d = nc.dram_tensor("WOUT", [128, 8], DT.bfloat16, kind="ExternalInput")
    idt_d = nc.dram_tensor("IDT", [128, 128], DT.bfloat16, kind="ExternalInput")
    ht_d = nc.dram_tensor("HT", [128, 8, 128], DT.bfloat16, kind="ExternalInput")
    cs_d = nc.dram_tensor("CS", [128, 2, 512], DT.float32, kind="ExternalInput")
    bout_d = nc.dram_tensor("BOUT", [1, 1], DT.float32, kind="ExternalInput")
    xinit_d = nc.dram_tensor("XINIT", [2, BL], DT.bfloat16, kind="ExternalInput")
    outd = nc.dram_tensor("OUTD", [T_FULL, BL], DT.float32, kind="ExternalOutput")

    with tile.TileContext(nc) as tc:
        with (
            tc.tile_pool(name="const", bufs=1) as const,
            tc.tile_pool(name="psum", bufs=8, space="PSUM") as psum,
            tc.tile_pool(name="tmp", bufs=3) as tmp,
            tc.tile_pool(name="drow", bufs=2) as drowp,
        ):
            w0 = const.tile([128, 9, 4096], DT.bfloat16)
            w1 = const.tile([128, 17, 4096], DT.bfloat16)
            wout = const.tile([128, 8], DT.bfloat16)
            idt = const.tile([128, 128], DT.bfloat16)
            ht = const.tile([128, 8, 128], DT.bfloat16)
            cs = const.tile([128, 2, 512], DT.float32)
            bout = const.tile([1, 1], DT.float32)
            xch = const.tile([2, BL], DT.bfloat16)
            ones = const.tile([1, BL], DT.bfloat16)
            hs0 = const.tile([128, 512], DT.bfloat16)
            hs1 = const.tile([128, 512], DT.bfloat16)

            nc.sync.dma_start(w0[:], w0_d[:])
            nc.sync.dma_start(w1[:], w1_d[:])
            nc.sync.dma_start(wout[:], wout_d[:])
            nc.sync.dma_start(idt[:], idt_d[:])
            nc.sync.dma_start(ht[:], ht_d[:])
            nc.sync.dma_start(cs[:], cs_d[:])
            nc.sync.dma_start(bout[:], bout_d[:])

            nc.sync.dma_start(xch[:], xinit_d[:])
            nc.vector.memset(ones[:], 1.0)

            # h^T chunk APs: layer-l chunk k covers hidden dims [128k, 128k+128)
            # block p of ht holds cols 0:64 = chunk p, cols 64:128 = chunk 4+p.
            def h_chunk(layer, k):
                base = 4 * layer
                if k < 4:
                    return ht[:, base + k, 0:BL]
                return ht[:, base + (k - 4), BL : 2 * BL]

            GFUNC = [AF.Sigmoid, AF.Sigmoid, AF.Tanh, AF.Sigmoid]

            for t in range(t_steps):
                # ---- L0 gate matmuls: gates0 = [h0; x, 1] @ [Whh0; Wih0, b0]
                p0 = [psum.tile([128, 512], DT.float32, tag="bank", name=f"p0_{t}_{g}") for g in range(4)]
                for k in range(9):
                    lhsT = h_chunk(0, k) if k < 8 else xch[0:2, :]
                    for g in range(4):
                        for hf in range(2):
                            s = 1024 * g + 512 * hf
                            rhs = (
                                w0[:, k, s : s + 512]
                                if k < 8
                                else w0[0:2, 8, s : s + 512]
                            )
                            nc.tensor.matmul(
                                p0[g][BL * hf : BL * (hf + 1), :],
                                lhsT,
                                rhs,
                                start=(k == 0),
                                stop=(k == 8),
                            )

                # ---- L1 early part: h1(t-1) @ Whh1 + bias (independent of L0)
                p1 = [psum.tile([128, 512], DT.float32, tag="bank", name=f"p1_{t}_{g}") for g in range(4)]
                for k in range(8):
                    lhsT = h_chunk(1, k)
                    for g in range(4):
                        for hf in range(2):
                            s = 1024 * g + 512 * hf
                            nc.tensor.matmul(
                                p1[g][BL * hf : BL * (hf + 1), :],
                                lhsT,
                                w1[:, 8 + k, s : s + 512],
                                start=(k == 0),
                                stop=False,
                            )
                for g in range(4):
                    for hf in range(2):
                        s = 1024 * g + 512 * hf
                        nc.tensor.matmul(
                            p1[g][BL * hf : BL * (hf + 1), :],
                            ones[0:1, :],
                            w1[0:1, 16, s : s + 512],
                            start=False,
                            stop=False,
                        )

                # ---- L0 activations + elementwise
                gs = tmp.tile([128, 512], DT.float32, tag="tmp")
                nc.scalar.activation(gs[:], p0[2][:], AF.Tanh)
                for g in (0, 1, 3):
                    nc.scalar.activation(p0[g][:], p0[g][:], AF.Sigmoid)
                t1 = tmp.tile([128, 512], DT.float32, tag="tmp")
                cs0 = cs[:, 0, :]
                nc.vector.tensor_mul(t1[:], p0[0][:], gs[:])
                nc.vector.tensor_mul(cs0, p0[1][:], cs0)
                nc.vector.tensor_add(cs0, cs0, t1[:])
                thc = tmp.tile([128, 512], DT.float32, tag="tmp")
                nc.scalar.activation(thc[:], cs0, AF.Tanh)
                nc.vector.tensor_mul(hs0[:], p0[3][:], thc[:])

                # ---- transpose h0 -> ht blocks 0..3
                trp0 = psum.tile([128, 4, 128], DT.bfloat16, tag="bank")
                for p in range(4):
                    nc.tensor.transpose(
                        trp0[:, p, :], hs0[:, 128 * p : 128 * (p + 1)], idt[:]
                    )
                for p in range(4):
                    nc.vector.tensor_copy(ht[:, p, :], trp0[:, p, :])

                # ---- L1 late part: h0(t) @ Wih1
                for k in range(8):
                    lhsT = h_chunk(0, k)
                    for g in range(4):
                        for hf in range(2):
                            s = 1024 * g + 512 * hf
                            nc.tensor.matmul(
                                p1[g][BL * hf : BL * (hf + 1), :],
                                lhsT,
                                w1[:, k, s : s + 512],
                                start=False,
                                stop=(k == 7),
                            )

                # ---- L1 activations + elementwise
                gs1 = tmp.tile([128, 512], DT.float32, tag="tmp")
                nc.scalar.activation(gs1[:], p1[2][:], AF.Tanh)
                for g in (0, 1, 3):
                    nc.scalar.activation(p1[g][:], p1[g][:], AF.Sigmoid)
                t2 = tmp.tile([128, 512], DT.float32, tag="tmp")
                cs1 = cs[:, 1, :]
                nc.vector.tensor_mul(t2[:], p1[0][:], gs1[:])
                nc.vector.tensor_mul(cs1, p1[1][:], cs1)
                nc.vector.tensor_add(cs1, cs1, t2[:])
                thc1 = tmp.tile([128, 512], DT.float32, tag="tmp")
                nc.scalar.activation(thc1[:], cs1, AF.Tanh)
                nc.vector.tensor_mul(hs1[:], p1[3][:], thc1[:])

                # ---- transpose h1 -> ht blocks 4..7
                trp1 = psum.tile([128, 4, 128], DT.bfloat16, tag="bank")
                for p in range(4):
                    nc.tensor.transpose(
                        trp1[:, p, :], hs1[:, 128 * p : 128 * (p + 1)], idt[:]
                    )
                for p in range(4):
                    nc.vector.tensor_copy(ht[:, 4 + p, :], trp1[:, p, :])

                # ---- out^T = W_out @ h1^T : [1, 64] via M=1 matmuls
                dps = psum.tile([1, BL], DT.float32, tag="bank")
                for k in range(8):
                    nc.tensor.matmul(
                        dps[0:1, :],
                        wout[:, k : k + 1],
                        h_chunk(1, k),
                        start=(k == 0),
                        stop=(k == 7),
                    )

                # ---- tail: out rows (fp32 to DRAM, bf16 to next x)
                drow = drowp.tile([1, BL], DT.float32, tag="drow")
                nc.scalar.activation(
                    drow[:], dps[0:1, :], AF.Identity, bias=bout[0:1, 0:1]
                )
                nc.scalar.activation(
                    xch[0:1, :], dps[0:1, :], AF.Identity, bias=bout[0:1, 0:1]
                )
                nc.sync.dma_start(outd[t : t + 1, :], drow[:])

    _split_multi_waits(nc)
    return nc


# ---------------------------------------------------------------------------
# host side


def _stack_batch(x):  # [64, 1024] -> stacked [128, 512]
    return x.reshape(BL, 2, 512).transpose(1, 0, 2).reshape(128, 512)


def _ht_blocks(h):  # [64, 1024] -> [128(j), 4(p), 128(r)]
    x = h.reshape(BL, 2, 4, 128)  # [b, half, p, j]
    return x.transpose(3, 2, 1, 0).reshape(128, 4, 128)


def _prep_shared(inp):
    w0 = np.zeros((128, 9, 4096), dtype=np.float32)
    w0[:, :8, :] = inp["W_hh0"].T.reshape(8, 128, 4096).transpose(1, 0, 2)
    w0[0, 8, :] = inp["W_ih0"][:, 0]
    w0[1, 8, :] = inp["b_ih0"] + inp["b_hh0"]

    w1 = np.zeros((128, 17, 4096), dtype=np.float32)
    w1[:, 0:8, :] = inp["W_ih1"].T.reshape(8, 128, 4096).transpose(1, 0, 2)
    w1[:, 8:16, :] = inp["W_hh1"].T.reshape(8, 128, 4096).transpose(1, 0, 2)
    w1[0, 16, :] = inp["b_ih1"] + inp["b_hh1"]

    wo = np.ascontiguousarray(inp["W_out"][0].reshape(8, 128).T)
    return {
        "W0": w0.astype(BF16),
        "W1": w1.astype(BF16),
        "WOUT": wo.astype(BF16),
        "IDT": np.eye(128, dtype=BF16),
        "XINIT": np.concatenate(
            [np.zeros((1, BL), BF16), np.ones((1, BL), BF16)], axis=0
        ),
        "BOUT": inp["b_out"].reshape(1, 1).astype(np.float32),
    }


def _prep_core(inp, c):
    sl = slice(BL * c, BL * (c + 1))
    ht = np.zeros((128, 8, 128), dtype=np.float32)
    ht[:, 0:4, :] = _ht_blocks(np.asarray(inp["h0"][0, sl]))
    ht[:, 4:8, :] = _ht_blocks(np.asarray(inp["h0"][1, sl]))
    cs = np.zeros((128, 2, 512), dtype=np.float32)
    cs[:, 0, :] = _stack_batch(np.asarray(inp["c0"][0, sl]))
    cs[:, 1, :] = _stack_batch(np.asarray(inp["c0"][1, sl]))
    return {"HT": ht.astype(BF16), "CS": cs}


_RUNNER = {}


def _get_runner(t_steps):
    """Build the bass program once per process and return a cached callable
    mapping per-core input dicts -> per-core OUTD arrays."""
    if t_steps in _RUNNER:
        return _RUNNER[t_steps]

    import jax
    from jax.sharding import Mesh, PartitionSpec
    from jax.experimental.shard_map import shard_map
    from concourse import bass2jax
    from concourse._compat import axon_active

    nc = _build_program(t_steps)

    if not axon_active():
        from concourse.bass_utils import run_bass_kernel_spmd

        def run_native(in_maps):
            res = run_bass_kernel_spmd(nc, in_maps, list(range(N_CORES)))
            return [r["OUTD"] for r in res.results]

        _RUNNER[t_steps] = run_native
        return run_native

    bass2jax.install_neuronx_cc_hook()

    partition_name = nc.partition_id_tensor.name if nc.partition_id_tensor else None
    in_names = []
    out_names = []
    out_avals = []
    zero_outs = []
    for alloc in nc.m.functions[0].allocations:
        if not isinstance(alloc, mybir.MemoryLocationSet):
            continue
        name = alloc.memorylocations[0].name
        if alloc.kind == "ExternalInput":
            if name != partition_name:
                in_names.append(name)
        elif alloc.kind == "ExternalOutput":
            out_names.append(name)
            shape = tuple(alloc.tensor_shape)
            dtype = mybir.dt.np(alloc.dtype)
            out_avals.append(jax.core.ShapedArray(shape, dtype))
            zero_outs.append(np.zeros(shape, dtype))
    n_params = len(in_names)
    n_outs = len(out_avals)
    all_names = in_names + out_names
    if partition_name is not None:
        all_names = all_names + [partition_name]
    donate = tuple(range(n_params, n_params + n_outs))

    def _body(*args):
        operands = list(args)
        if partition_name is not None:
            operands.append(bass2jax.partition_id_tensor())
        outs = bass2jax._bass_exec_p.bind(
            *operands,
            out_avals=tuple(out_avals),
            in_names=tuple(all_names),
            out_names=tuple(out_names),
            lowering_input_output_aliases=(),
            sim_require_finite=True,
            sim_require_nnan=True,
            nc=nc,
        )
        return tuple(outs)

    devices = jax.devices()[:N_CORES]
    mesh = Mesh(np.asarray(devices), ("core",))
    sharded = jax.jit(
        shard_map(
            _body,
            mesh=mesh,
            in_specs=(PartitionSpec("core"),) * (n_params + n_outs),
            out_specs=(PartitionSpec("core"),) * n_outs,
            check_rep=False,
        ),
        donate_argnums=donate,
        keep_unused=True,
    )

    def prep_args(in_maps):
        concat_in = [
            np.concatenate([np.asarray(in_maps[c][nm]) for c in range(N_CORES)], axis=0)
            for nm in in_names
        ]
        concat_zero = [np.concatenate([z] * N_CORES, axis=0) for z in zero_outs]
        return concat_in, concat_zero

    def run(in_maps):
        concat_in, concat_zero = prep_args(in_maps)
        out_arrs = sharded(*concat_in, *concat_zero)
        full = np.asarray(out_arrs[0])
        return np.split(full, N_CORES, axis=0)

    run.sharded = sharded
    run.prep_args = prep_args
    run.mesh = mesh
    _RUNNER[t_steps] = run
    return run


def kernel(**inputs):
    inp = {k: np.asarray(v) for k, v in inputs.items()}
    for k in ("W_ih0", "W_hh0", "b_ih0", "b_hh0", "W_ih1", "W_hh1", "b_ih1",
              "b_hh1", "W_out", "b_out", "h0", "c0", "outputs"):
        assert k in inp, f"missing input {k}"

    shared = _prep_shared(inp)
    in_maps = []
    for c in range(N_CORES):
        m = dict(shared)
        m.update(_prep_core(inp, c))
        in_maps.append(m)

    run = _get_runner(_T)
    outs = run(in_maps)  # list of [T_FULL, BL] fp32 per core

    out_all = np.concatenate(outs, axis=1)  # [T, B]
    targets = np.asarray(inp["outputs"]).T.astype(np.float32)  # [T, B]
    d = out_all[:_T].astype(np.float64) - targets[:_T].astype(np.float64)
    loss = np.sum(np.mean(d * d, axis=1))
    return np.float32(loss)


# revision 8
# speedup vs baseline: 2650.9101x; 3.6655x over previous
"""Trainium2 Bass kernel for a 2-layer LSTM decoder (B=512, T=128, H=1024).

Strategy (data-parallel over batch, per sharding hint):
  - 8 cores, 64 batch rows each; weights replicated, recurrence device-local.
  - All weights cast to bf16 and kept resident in SBUF for the whole T loop.
  - Gates are computed with batch as the matmul *stationary* operand
    (lhsT = h^T chunks, [K=128, M=64]) and weights streaming ([128, 512]).
    Two 512-wide gate chunks are computed concurrently via PE column tiling
    (outputs to PSUM partitions 0-63 and 64-127), giving a "stacked"
    [128, 512] gate tile layout that keeps all 128 lanes busy for the
    activation/elementwise work as well.
  - h is transposed back each step with PE-mode transposes (4x [128,128]).
  - The output projection is an 8-chunk matmul with M=1 producing out^T
    directly as a [1, 64] row, which feeds the next step's input chunk and
    is DMA'd to DRAM; the MSE-loss tail is computed on the host in fp32.
"""

import os

import numpy as np
import ml_dtypes

import concourse.bass as bass
import concourse.tile as tile
import concourse.mybir as mybir

BF16 = ml_dtypes.bfloat16
N_CORES = 8
B, T_FULL, H = 512, 128, 1024
BL = B // N_CORES  # 64 local batch rows
AF = mybir.ActivationFunctionType
DT = mybir.dt

_T = int(os.environ.get("LSTM_KERNEL_T", str(T_FULL)))


def _split_multi_waits(nc):
    """walrus in this container supports only ONE sync wait per instruction.
    Move extra waits onto preceding same-engine NOPs (engine FIFO makes this
    semantically identical)."""
    for f in nc.m.functions:
        for bb in f.blocks:
            new = []
            for ins in bb.instructions:
                si = ins.sync_info
                if si is not None and si.on_wait and len(si.on_wait) > 1:
                    waits = list(si.on_wait)
                    for w in waits[:-1]:
                        nop = mybir.InstNoOp(
                            name=nc.get_next_instruction_name(), ins=[], outs=[]
                        )
                        nop.engine = ins.engine
                        nop.sync_info = mybir.SyncInfo(on_wait=[w], on_update=[])
                        nc.register_instruction(nop)
                        new.append(nop)
                    si.on_wait = [waits[-1]]
                new.append(ins)
            bb.instructions = new


def _build_program(t_steps):
    nc = bass.Bass(dynamic_dma_scratch_size=512)

    w0_d = nc.dram_tensor("W0", [128, 9, 4096], DT.bfloat16, kind="ExternalInput")
    w1_d = nc.dram_tensor("W1", [128, 17, 4096], DT.bfloat16, kind="ExternalInput")
    wstack_d = nc.dram_tensor("WSTACK", [128, 512], DT.bfloat16, kind="ExternalInput")
    fold_d = nc.dram_tensor("FOLD", [128, BL], DT.float32, kind="ExternalInput")
    idt_d = nc.dram_tensor("IDT", [128, 128], DT.bfloat16, kind="ExternalInput")
    ht_d = nc.dram_tensor("HT", [128, 8, 128], DT.bfloat16, kind="ExternalInput")
    cs_d = nc.dram_tensor("CS", [128, 2, 512], DT.float32, kind="ExternalInput")
    bout_d = nc.dram_tensor("BOUT", [1, 1], DT.float32, kind="ExternalInput")
    xinit_d = nc.dram_tensor("XINIT", [2, BL], DT.bfloat16, kind="ExternalInput")
    outd = nc.dram_tensor("OUTD", [T_FULL, BL], DT.float32, kind="ExternalOutput")

    with tile.TileContext(nc) as tc:
        with (
            tc.tile_pool(name="const", bufs=1) as const,
            tc.tile_pool(name="psum", bufs=8, space="PSUM") as psum,
            tc.tile_pool(name="tmp", bufs=2) as tmp,
            tc.tile_pool(name="drow", bufs=2) as drowp,
        ):
            w0 = const.tile([128, 9, 4096], DT.bfloat16)
            w1 = const.tile([128, 17, 4096], DT.bfloat16)
            wstack = const.tile([128, 512], DT.bfloat16)
            fold = const.tile([128, BL], DT.float32)
            part = const.tile([128, 1], DT.float32)
            idt = const.tile([128, 128], DT.bfloat16)
            ht = const.tile([128, 8, 128], DT.bfloat16)
            cs = const.tile([128, 2, 512], DT.float32)
            bout = const.tile([1, 1], DT.float32)
            xch = const.tile([2, BL], DT.bfloat16)
            ones = const.tile([1, BL], DT.bfloat16)
            hs0 = const.tile([128, 512], DT.bfloat16)
            hs1 = const.tile([128, 512], DT.bfloat16)

            nc.sync.dma_start(w0[:], w0_d[:])
            nc.sync.dma_start(w1[:], w1_d[:])
            nc.sync.dma_start(wstack[:], wstack_d[:])
            nc.sync.dma_start(fold[:], fold_d[:])
            nc.sync.dma_start(idt[:], idt_d[:])
            nc.sync.dma_start(ht[:], ht_d[:])
            nc.sync.dma_start(cs[:], cs_d[:])
            nc.sync.dma_start(bout[:], bout_d[:])

            nc.sync.dma_start(xch[:], xinit_d[:])
            nc.vector.memset(ones[:], 1.0)

            # h^T chunk APs: layer-l chunk k covers hidden dims [128k, 128k+128)
            # block p of ht holds cols 0:64 = chunk p, cols 64:128 = chunk 4+p.
            def h_chunk(layer, k):
                base = 4 * layer
                if k < 4:
                    return ht[:, base + k, 0:BL]
                return ht[:, base + (k - 4), BL : 2 * BL]

            GFUNC = [AF.Sigmoid, AF.Sigmoid, AF.Tanh, AF.Sigmoid]

            for t in range(t_steps):
                # ---- L0 gate matmuls: gates0 = [h0; x, 1] @ [Whh0; Wih0, b0]
                p0 = [psum.tile([128, 512], DT.float32, tag="bank", name=f"p0_{t}_{g}") for g in range(4)]
                for k in range(9):
                    lhsT = h_chunk(0, k) if k < 8 else xch[0:2, :]
                    for g in range(4):
                        for hf in range(2):
                            s = 1024 * g + 512 * hf
                            rhs = (
                                w0[:, k, s : s + 512]
                                if k < 8
                                else w0[0:2, 8, s : s + 512]
                            )
                            nc.tensor.matmul(
                                p0[g][BL * hf : BL * (hf + 1), :],
                                lhsT,
                                rhs,
                                start=(k == 0),
                                stop=(k == 8),
                            )

                # ---- L1 early part: h1(t-1) @ Whh1 + bias (independent of L0)
                p1 = [psum.tile([128, 512], DT.float32, tag="bank", name=f"p1_{t}_{g}") for g in range(4)]
                for k in range(8):
                    lhsT = h_chunk(1, k)
                    for g in range(4):
                        for hf in range(2):
                            s = 1024 * g + 512 * hf
                            nc.tensor.matmul(
                                p1[g][BL * hf : BL * (hf + 1), :],
                                lhsT,
                                w1[:, 8 + k, s : s + 512],
                                start=(k == 0),
                                stop=False,
                            )
                for g in range(4):
                    for hf in range(2):
                        s = 1024 * g + 512 * hf
                        nc.tensor.matmul(
                            p1[g][BL * hf : BL * (hf + 1), :],
                            ones[0:1, :],
                            w1[0:1, 16, s : s + 512],
                            start=False,
                            stop=False,
                        )

                # ---- L0 activations + elementwise
                gs = tmp.tile([128, 512], DT.float32, tag="tmp")
                nc.scalar.activation(gs[:], p0[2][:], AF.Tanh)
                for g in (0, 1, 3):
                    nc.scalar.activation(p0[g][:], p0[g][:], AF.Sigmoid)
                t1 = tmp.tile([128, 512], DT.float32, tag="tmp")
                cs0 = cs[:, 0, :]
                nc.vector.tensor_mul(t1[:], p0[0][:], gs[:])
                nc.vector.tensor_mul(cs0, p0[1][:], cs0)
                nc.vector.tensor_add(cs0, cs0, t1[:])
                thc = tmp.tile([128, 512], DT.float32, tag="tmp")
                nc.scalar.activation(thc[:], cs0, AF.Tanh)
                nc.vector.tensor_mul(hs0[:], p0[3][:], thc[:])

                # ---- transpose h0 -> ht blocks 0..3
                trp0 = psum.tile([128, 4, 128], DT.bfloat16, tag="bank")
                for p in range(4):
                    nc.tensor.transpose(
                        trp0[:, p, :], hs0[:, 128 * p : 128 * (p + 1)], idt[:]
                    )
                nc.vector.tensor_copy(ht[:, 0:4, :], trp0[:, :, :])

                # ---- L1 late part: h0(t) @ Wih1
                for k in range(8):
                    lhsT = h_chunk(0, k)
                    for g in range(4):
                        for hf in range(2):
                            s = 1024 * g + 512 * hf
                            nc.tensor.matmul(
                                p1[g][BL * hf : BL * (hf + 1), :],
                                lhsT,
                                w1[:, k, s : s + 512],
                                start=False,
                                stop=(k == 7),
                            )

                # ---- L1 activations + elementwise
                gs1 = tmp.tile([128, 512], DT.float32, tag="tmp")
                nc.scalar.activation(gs1[:], p1[2][:], AF.Tanh)
                for g in (0, 1, 3):
                    nc.scalar.activation(p1[g][:], p1[g][:], AF.Sigmoid)
                t2 = tmp.tile([128, 512], DT.float32, tag="tmp")
                cs1 = cs[:, 1, :]
                nc.vector.tensor_mul(t2[:], p1[0][:], gs1[:])
                nc.vector.tensor_mul(cs1, p1[1][:], cs1)
                nc.vector.tensor_add(cs1, cs1, t2[:])
                thc1 = tmp.tile([128, 512], DT.float32, tag="tmp")
                nc.scalar.activation(thc1[:], cs1, AF.Tanh)
                nc.vector.tensor_mul(hs1[:], p1[3][:], thc1[:])

                # ---- transpose h1 -> ht blocks 4..7 (DMA xbar, off PE)
                for p in range(4):
                    nc.scalar.dma_start_transpose(
                        ht[:, 4 + p, :], hs1[:, 128 * p : 128 * (p + 1)]
                    )

                # ---- out = W_out . h1 : fused DVE mul-reduce + fold matmul
                nc.vector.tensor_mul(thc1[:], hs1[:], wstack[:])
                nc.vector.tensor_reduce(
                    out=part[:, 0:1],
                    in_=thc1[:],
                    op=mybir.AluOpType.add,
                    axis=mybir.AxisListType.X,
                )
                dps = psum.tile([1, BL], DT.float32, tag="bank")
                nc.tensor.matmul(dps[0:1, :], part[:, 0:1], fold[:], start=True, stop=True)

                # ---- tail: out rows (fp32 to DRAM, bf16 to next x)
                drow = drowp.tile([1, BL], DT.float32, tag="drow")
                nc.scalar.activation(
                    drow[:], dps[0:1, :], AF.Identity, bias=bout[0:1, 0:1]
                )
                nc.scalar.activation(
                    xch[0:1, :], dps[0:1, :], AF.Identity, bias=bout[0:1, 0:1]
                )
                nc.sync.dma_start(outd[t : t + 1, :], drow[:])

    _split_multi_waits(nc)
    return nc


# ---------------------------------------------------------------------------
# host side


def _stack_batch(x):  # [64, 1024] -> stacked [128, 512]
    return x.reshape(BL, 2, 512).transpose(1, 0, 2).reshape(128, 512)


def _ht_blocks(h):  # [64, 1024] -> [128(j), 4(p), 128(r)]
    x = h.reshape(BL, 2, 4, 128)  # [b, half, p, j]
    return x.transpose(3, 2, 1, 0).reshape(128, 4, 128)


def _prep_shared(inp):
    w0 = np.zeros((128, 9, 4096), dtype=np.float32)
    w0[:, :8, :] = inp["W_hh0"].T.reshape(8, 128, 4096).transpose(1, 0, 2)
    w0[0, 8, :] = inp["W_ih0"][:, 0]
    w0[1, 8, :] = inp["b_ih0"] + inp["b_hh0"]

    w1 = np.zeros((128, 17, 4096), dtype=np.float32)
    w1[:, 0:8, :] = inp["W_ih1"].T.reshape(8, 128, 4096).transpose(1, 0, 2)
    w1[:, 8:16, :] = inp["W_hh1"].T.reshape(8, 128, 4096).transpose(1, 0, 2)
    w1[0, 16, :] = inp["b_ih1"] + inp["b_hh1"]

    wstack = inp["W_out"][0].reshape(2, 512)[None, :, :].repeat(BL, axis=0)
    wstack = wstack.transpose(1, 0, 2).reshape(128, 512)
    foldm = np.zeros((128, BL), dtype=np.float32)
    for p in range(128):
        foldm[p, p % BL] = 1.0
    return {
        "W0": w0.astype(BF16),
        "W1": w1.astype(BF16),
        "WSTACK": wstack.astype(BF16),
        "FOLD": foldm,
        "IDT": np.eye(128, dtype=BF16),
        "XINIT": np.concatenate(
            [np.zeros((1, BL), BF16), np.ones((1, BL), BF16)], axis=0
        ),
        "BOUT": inp["b_out"].reshape(1, 1).astype(np.float32),
    }


def _prep_core(inp, c):
    sl = slice(BL * c, BL * (c + 1))
    ht = np.zeros((128, 8, 128), dtype=np.float32)
    ht[:, 0:4, :] = _ht_blocks(np.asarray(inp["h0"][0, sl]))
    ht[:, 4:8, :] = _ht_blocks(np.asarray(inp["h0"][1, sl]))
    cs = np.zeros((128, 2, 512), dtype=np.float32)
    cs[:, 0, :] = _stack_batch(np.asarray(inp["c0"][0, sl]))
    cs[:, 1, :] = _stack_batch(np.asarray(inp["c0"][1, sl]))
    return {"HT": ht.astype(BF16), "CS": cs}


_RUNNER = {}


def _get_runner(t_steps):
    """Build the bass program once per process and return a cached callable
    mapping per-core input dicts -> per-core OUTD arrays."""
    if t_steps in _RUNNER:
        return _RUNNER[t_steps]

    import jax
    from jax.sharding import Mesh, PartitionSpec
    from jax.experimental.shard_map import shard_map
    from concourse import bass2jax
    from concourse._compat import axon_active

    nc = _build_program(t_steps)

    if not axon_active():
        from concourse.bass_utils import run_bass_kernel_spmd

        def run_native(in_maps):
            res = run_bass_kernel_spmd(nc, in_maps, list(range(N_CORES)))
            return [r["OUTD"] for r in res.results]

        _RUNNER[t_steps] = run_native
        return run_native

    bass2jax.install_neuronx_cc_hook()

    partition_name = nc.partition_id_tensor.name if nc.partition_id_tensor else None
    in_names = []
    out_names = []
    out_avals = []
    zero_outs = []
    for alloc in nc.m.functions[0].allocations:
        if not isinstance(alloc, mybir.MemoryLocationSet):
            continue
        name = alloc.memorylocations[0].name
        if alloc.kind == "ExternalInput":
            if name != partition_name:
                in_names.append(name)
        elif alloc.kind == "ExternalOutput":
            out_names.append(name)
            shape = tuple(alloc.tensor_shape)
            dtype = mybir.dt.np(alloc.dtype)
            out_avals.append(jax.core.ShapedArray(shape, dtype))
            zero_outs.append(np.zeros(shape, dtype))
    n_params = len(in_names)
    n_outs = len(out_avals)
    all_names = in_names + out_names
    if partition_name is not None:
        all_names = all_names + [partition_name]
    donate = tuple(range(n_params, n_params + n_outs))

    def _body(*args):
        operands = list(args)
        if partition_name is not None:
            operands.append(bass2jax.partition_id_tensor())
        outs = bass2jax._bass_exec_p.bind(
            *operands,
            out_avals=tuple(out_avals),
            in_names=tuple(all_names),
            out_names=tuple(out_names),
            lowering_input_output_aliases=(),
            sim_require_finite=True,
            sim_require_nnan=True,
            nc=nc,
        )
        return tuple(outs)

    devices = jax.devices()[:N_CORES]
    mesh = Mesh(np.asarray(devices), ("core",))
    sharded = jax.jit(
        shard_map(
            _body,
            mesh=mesh,
            in_specs=(PartitionSpec("core"),) * (n_params + n_outs),
            out_specs=(PartitionSpec("core"),) * n_outs,
            check_rep=False,
        ),
        donate_argnums=donate,
        keep_unused=True,
    )

    def prep_args(in_maps):
        concat_in = [
            np.concatenate([np.asarray(in_maps[c][nm]) for c in range(N_CORES)], axis=0)
            for nm in in_names
        ]
        concat_zero = [np.concatenate([z] * N_CORES, axis=0) for z in zero_outs]
        return concat_in, concat_zero

    def run(in_maps):
        concat_in, concat_zero = prep_args(in_maps)
        out_arrs = sharded(*concat_in, *concat_zero)
        full = np.asarray(out_arrs[0])
        return np.split(full, N_CORES, axis=0)

    run.sharded = sharded
    run.prep_args = prep_args
    run.mesh = mesh
    _RUNNER[t_steps] = run
    return run


def kernel(**inputs):
    inp = {k: np.asarray(v) for k, v in inputs.items()}
    for k in ("W_ih0", "W_hh0", "b_ih0", "b_hh0", "W_ih1", "W_hh1", "b_ih1",
              "b_hh1", "W_out", "b_out", "h0", "c0", "outputs"):
        assert k in inp, f"missing input {k}"

    shared = _prep_shared(inp)
    in_maps = []
    for c in range(N_CORES):
        m = dict(shared)
        m.update(_prep_core(inp, c))
        in_maps.append(m)

    run = _get_runner(_T)
    outs = run(in_maps)  # list of [T_FULL, BL] fp32 per core

    out_all = np.concatenate(outs, axis=1)  # [T, B]
    targets = np.asarray(inp["outputs"]).T.astype(np.float32)  # [T, B]
    d = out_all[:_T].astype(np.float64) - targets[:_T].astype(np.float64)
    loss = np.sum(np.mean(d * d, axis=1))
    return np.float32(loss)
